# revision 13
# baseline (speedup 1.0000x reference)
"""GCN layer on 8 Trainium2 NeuronCores (Bass/Tile, SPMD).

  H = X @ W.T  (bias folded in later);  out[r] = sum_{e: row[e]=r} val[e] * H[col[e]] + rowsum_val[r] * b

Sharding: destination nodes (rows of `out`) are split into 8 contiguous
shards of 12500. Edges are partitioned by destination shard on the host, so
each core's segment-sum is local. Every core computes the full projection H
(replicated; avoids any collective) into a DRAM scratch, then aggregates its
own edges with token-gather DMAs (gpsimd.dma_gather) of H rows plus
"staircase" one-hot matmuls that accumulate the segment sums in PSUM. The
per-destination bias term sum(val)*b is applied as a rank-1 (K=1) matmul per
128-row block.

dma_gather indices are int16, so H is addressed in 4 banks of 32768 rows;
each core's edge stream is organized as groups of GRP_BLK destination blocks,
bank-major within a group, so one dma_gather instruction covers all of a
group's edges for one bank.

All index preprocessing (edge sort, block/bank padding, chunk layout) happens
on the host inside kernel(); the device program is identical across cores
(SPMD) — only DRAM contents differ.
"""

import math
import os

import ml_dtypes
import numpy as np

import concourse.bacc as bacc
import concourse.bass as bass
import concourse.mybir as mybir
import concourse.tile as tile
from concourse.bass_utils import run_bass_kernel_spmd

P = 128            # partitions / tile edge
N_NODES = 100000
IN_DIM = 256
OUT_DIM = 128
N_CORES = 8
SHARD = N_NODES // N_CORES          # 12500 destination rows per core
N_BLK = math.ceil(SHARD / P)        # 98 blocks of 128 dest rows (last = 84)
NODE_PAD = 100352                   # nodes padded to 196*512 for H compute
BANK = 32768                        # H rows per gather bank (int16 idx limit)
N_BANK = math.ceil(NODE_PAD / BANK)  # 4
GRP_BLK = 4                         # dest blocks per PSUM group (1 PSUM bank)

F32 = mybir.dt.float32
BF16 = mybir.dt.bfloat16
I16 = mybir.dt.int16

LAST_RESULTS = None  # BassKernelResults of the most recent run (for test.py)


def _install_ntff_shim():
    """Provide antenv.axon_hooks (absent in this image) so that
    run_bass_kernel_spmd(trace=True) can capture NTFF profiles through the
    axon PJRT .so. Mirrors trn_agent_boot.trn_boot._ntff_profile_via_ctypes.
    Only used for local profiling runs (GCN_TRACE=1); the plain path never
    imports it."""
    import contextlib
    import ctypes
    import sys
    import types

    if "antenv.axon_hooks" in sys.modules:
        return
    try:
        lib = ctypes.CDLL("/opt/axon/libaxon_pjrt.so")
    except OSError:
        return
    if not hasattr(lib, "axon_start_nrt_profile"):
        return
    lib.axon_start_nrt_profile.argtypes = [
        ctypes.POINTER(ctypes.c_int64), ctypes.c_size_t]
    lib.axon_start_nrt_profile.restype = ctypes.c_int64
    lib.axon_stop_nrt_profile.argtypes = [ctypes.c_char_p]
    lib.axon_stop_nrt_profile.restype = ctypes.c_int64

    @contextlib.contextmanager
    def _hook(output_dir, device_ids):
        import jax
        jax.devices()
        if device_ids:
            ids = (ctypes.c_int64 * len(device_ids))(*device_ids)
            rc = lib.axon_start_nrt_profile(ids, len(device_ids))
        else:
            rc = lib.axon_start_nrt_profile(None, 0)
        if rc != 0:
            raise RuntimeError(f"axon_start_nrt_profile rc={rc}")
        try:
            yield
        finally:
            n = lib.axon_stop_nrt_profile(str(output_dir).encode())
            print(f"ntff profile: {n} file(s) written to {output_dir}")

    mod = types.ModuleType("antenv.axon_hooks")
    mod.get_axon_ntff_profile_hook = lambda: _hook
    mod.set_axon_ntff_profile_hook = lambda h: None
    sys.modules["antenv.axon_hooks"] = mod

    import concourse.bass_utils as bu
    bu.upload_artifacts = lambda tmpdir: f"local://{tmpdir}"


def _host_prep(edge_row, edge_col, edge_val):
    """Partition edges by (core, block, bank), pad each cell to a uniform
    (across cores) multiple-of-128 chunk count, and lay out per-core arrays.

    Chunk stream order (identical across cores): for each group of GRP_BLK
    blocks, for each bank, for each block in the group, that cell's chunks.

    Returns (per_core, layout) where per_core[c] = (idx16, lrowf, valf, rs)
    and layout = dict(chunk_block, segs) with segs = list of
    (bank, t0, t1) gather segments in stream order.
    """
    er = np.asarray(edge_row, dtype=np.int64)
    ec = np.asarray(edge_col, dtype=np.int64)
    ev = np.asarray(edge_val, dtype=np.float32)

    bank = ec // BANK
    core = er // SHARD
    rl = er - core * SHARD          # local dest row within shard
    lb = rl // P                    # local block
    lrow = rl - lb * P              # dest row within block

    # sort by (core, block, bank); within cell any order
    key = (core * N_BLK + lb) * N_BANK + bank
    order = np.argsort(key, kind="stable")
    er, ec, ev, key = er[order], ec[order], ev[order], key[order]
    rl, lb, lrow, bank, core = (rl[order], lb[order], lrow[order],
                                bank[order], core[order])

    # per-(core, block, bank) counts -> uniform chunk counts C[b, k]
    cnt = np.bincount(key, minlength=N_CORES * N_BLK * N_BANK).reshape(
        N_CORES, N_BLK, N_BANK)
    C = np.maximum(1, -(-cnt.max(axis=0) // P))     # [N_BLK, N_BANK]

    n_grp = -(-N_BLK // GRP_BLK)
    # stream order: group g -> bank k -> block b in group
    chunk_block = []
    chunk_bank = []
    segs = []            # (bank, t0, t1) per gather instruction
    cell_t0 = np.zeros((N_BLK, N_BANK), dtype=np.int64)  # chunk offset of cell
    t = 0
    for g in range(n_grp):
        b_lo, b_hi = g * GRP_BLK, min((g + 1) * GRP_BLK, N_BLK)
        for k in range(N_BANK):
            t0 = t
            for b in range(b_lo, b_hi):
                cell_t0[b, k] = t
                c_bk = int(C[b, k])
                chunk_block.extend([b] * c_bk)
                chunk_bank.extend([k] * c_bk)
                t += c_bk
            segs.append((k, t0, t))
    t_ch = t
    chunk_block = np.asarray(chunk_block)
    t_edges = P * t_ch

    per_core = []
    edge_cell = lb * N_BANK + bank          # cell id within core [N_BLK*N_BANK]
    for c in range(N_CORES):
        s = int(np.searchsorted(core, c, side="left"))
        e = int(np.searchsorted(core, c, side="right"))
        n_c = e - s
        cell = edge_cell[s:e]
        # rank within cell (edges sorted by cell)
        cnt_c = cnt[c].reshape(-1)
        idx0 = np.concatenate([[0], np.cumsum(cnt_c)[:-1]])
        rank = np.arange(n_c) - np.repeat(idx0, cnt_c)
        pos = np.repeat(cell_t0.reshape(-1) * P, cnt_c) + rank

        col_pad = np.zeros(t_edges, dtype=np.int64)
        val_pad = np.zeros(t_edges, dtype=np.float32)
        lrow_pad = np.zeros(t_edges, dtype=np.float32)
        colloc = ec[s:e] - bank[s:e] * BANK     # bank-local row id
        col_pad[pos] = colloc
        val_pad[pos] = ev[s:e]
        lrow_pad[pos] = lrow[s:e]

        # [t_ch, 128] -> [128, t_ch]: tile[p, t] = edge t*128+p
        valf = np.ascontiguousarray(val_pad.reshape(t_ch, P).T)
        lrowf = np.ascontiguousarray(lrow_pad.reshape(t_ch, P).T)

        # int16 gather indices: per segment, position i at [i%16, i//16],
        # replicated across the 8 16-partition stripes. Global layout: the
        # int16 tile is [16, t_edges//16] with edge (128*t0 + i) of segment
        # (t0,t1) at column t0*8 + i//16 — equivalently, edge j (global
        # stream position) at [j%16, j//16] since segments are contiguous
        # multiples of 128 edges.
        i16 = np.zeros((16, t_edges // 16), dtype=np.int16)
        j = np.arange(t_edges)
        i16[j % 16, j // 16] = col_pad
        idx16 = np.ascontiguousarray(np.tile(i16, (8, 1)))

        rs = np.zeros(N_BLK * P, dtype=np.float32)
        np.add.at(rs, rl[s:e].astype(np.int64), ev[s:e])
        per_core.append((idx16, lrowf, valf, rs[None, :]))

    # per-block ordered chunk list [(bank, t)] for block-contiguous matmuls
    chunk_bank = np.asarray(chunk_bank)
    block_chunks = [[] for _ in range(N_BLK)]
    for t in range(t_ch):
        block_chunks[int(chunk_block[t])].append((int(chunk_bank[t]), t))

    layout = dict(chunk_block=chunk_block, segs=segs, t_ch=t_ch,
                  block_chunks=block_chunks)
    return per_core, layout


def _build_program(layout, n_nodes_pad=NODE_PAD, shard=SHARD,
                   n_blk=N_BLK, out_dim=OUT_DIM, in_dim=IN_DIM):
    """Build the SPMD Bass program. Identical for all cores."""
    t_ch = layout["t_ch"]
    chunk_block = layout["chunk_block"]
    segs = layout["segs"]

    nc = bacc.Bacc("TRN2", target_bir_lowering=False, debug=False,
                   num_devices=N_CORES, num_swdge_queues=4)

    xt_d = nc.dram_tensor("XT", [in_dim, n_nodes_pad], BF16, kind="ExternalInput")
    wt_d = nc.dram_tensor("WT", [in_dim, out_dim], BF16, kind="ExternalInput")
    b_d = nc.dram_tensor("BROW", [1, out_dim], F32, kind="ExternalInput")
    iota_d = nc.dram_tensor("IOTA", [P, P], F32, kind="ExternalInput")
    idx_d = nc.dram_tensor("IDX16", [P, t_ch * 8], I16, kind="ExternalInput")
    lrow_d = nc.dram_tensor("LROW", [P, t_ch], F32, kind="ExternalInput")
    val_d = nc.dram_tensor("VAL", [P, t_ch], F32, kind="ExternalInput")
    rs_d = nc.dram_tensor("RS", [1, n_blk * P], F32, kind="ExternalInput")
    out_d = nc.dram_tensor("OUT", [shard, out_dim], F32, kind="ExternalOutput")

    n_win = n_nodes_pad // 512          # phase-A windows of 512 nodes
    k_tiles = in_dim // P               # contraction tiles (2)
    max_seg_ch = max(t1 - t0 for _, t0, t1 in segs)

    with tile.TileContext(nc) as tc:
        with (
            tc.tile_pool(name="dram", bufs=1, space="DRAM") as dram,
            tc.tile_pool(name="consts", bufs=1) as consts,
            tc.tile_pool(name="meta", bufs=1) as meta,
        ):
            h_dram = dram.tile([n_nodes_pad, out_dim], F32)

            # constants
            wt_s = consts.tile([P, k_tiles, out_dim], BF16)
            nc.sync.dma_start(
                out=wt_s[:],
                in_=wt_d[:].rearrange("(k p) o -> p k o", p=P),
            )
            iota_s = consts.tile([P, P], F32)
            nc.sync.dma_start(out=iota_s[:], in_=iota_d[:])
            b_s = consts.tile([1, out_dim], F32)
            nc.sync.dma_start(out=b_s[:], in_=b_d[:])
            rs_s = consts.tile([1, n_blk * P], F32)
            nc.sync.dma_start(out=rs_s[:], in_=rs_d[:])

            # all chunk metadata up front (overlaps phase A)
            idx_s = meta.tile([P, t_ch * 8], I16)
            lrow_s = meta.tile([P, t_ch], F32)
            val_s = meta.tile([P, t_ch], F32)
            nc.scalar.dma_start(out=idx_s[:], in_=idx_d[:])
            nc.scalar.dma_start(out=lrow_s[:], in_=lrow_d[:])
            nc.scalar.dma_start(out=val_s[:], in_=val_d[:])

            # ---- Phase A: H = X @ W.T (replicated, all nodes) ----
            with (
                tc.tile_pool(name="xa", bufs=3) as xa,
                tc.tile_pool(name="ha", bufs=3) as ha,
                tc.tile_pool(name="psa", bufs=3, space="PSUM") as psa,
            ):
                for w in range(n_win):
                    n0 = w * 512
                    xt_t = xa.tile([P, k_tiles, 512], BF16)
                    nc.sync.dma_start(
                        out=xt_t[:],
                        in_=xt_d[:, n0:n0 + 512].rearrange(
                            "(k p) n -> p k n", p=P),
                    )
                    h_t = ha.tile([P, 4, out_dim], F32)
                    pst = psa.tile([P, 4, out_dim], F32, space="PSUM")
                    for s in range(4):
                        for k in range(k_tiles):
                            nc.tensor.matmul(
                                out=pst[:, s, :],
                                lhsT=xt_t[:, k, s * P:(s + 1) * P],
                                rhs=wt_s[:, k, :],
                                start=(k == 0),
                                stop=(k == k_tiles - 1),
                            )
                    nc.scalar.activation(
                        h_t[:], pst[:],
                        mybir.ActivationFunctionType.Copy)
                    nc.sync.dma_start(
                        out=h_dram[n0:n0 + 512, :].rearrange(
                            "(s p) o -> p s o", p=P),
                        in_=h_t[:],
                    )

            # ---- Phase B: bank gathers + staircase segment-sum ----
            with (
                tc.tile_pool(name="hg", bufs=2 * N_BANK) as hgp,
                tc.tile_pool(name="st", bufs=8) as stp,
                tc.tile_pool(name="ot", bufs=3) as otp,
                tc.tile_pool(name="psb", bufs=3, space="PSUM") as psb,
            ):
                block_chunks = layout["block_chunks"]
                n_grp = -(-n_blk // GRP_BLK)
                seg_idx = 0
                for g in range(n_grp):
                    blk0 = g * GRP_BLK
                    psum_g = psb.tile([P, GRP_BLK, out_dim], F32,
                                      space="PSUM", name="ps_g")
                    # gather this group's edges, one instruction per bank
                    hg_tiles = []
                    seg_t0 = []
                    for k in range(N_BANK):
                        kk, t0, t1 = segs[seg_idx]
                        seg_idx += 1
                        assert kk == k
                        n_ch = t1 - t0
                        bk_lo = k * BANK
                        bk_hi = min((k + 1) * BANK, n_nodes_pad)
                        hg = hgp.tile([P, max_seg_ch, P], F32, name="hg")
                        # SWDGE descriptor carveout limits one gather to 1024
                        # descriptors -> split into sub-gathers of <= 8 chunks
                        for q0 in range(0, n_ch, 8):
                            q1 = min(q0 + 8, n_ch)
                            nc.gpsimd.dma_gather(
                                out_ap=hg[:, q0:q1, :],
                                in_ap=h_dram[bk_lo:bk_hi, :],
                                idxs_ap=idx_s[:, (t0 + q0) * 8:(t0 + q1) * 8],
                                num_idxs=(q1 - q0) * P,
                                num_idxs_reg=(q1 - q0) * P,
                                elem_size=out_dim,
                                queue_num=(t0 + q0) % 4,
                            )
                        hg_tiles.append(hg)
                        seg_t0.append(t0)
                    # block-contiguous staircase accumulation
                    for bb in range(blk0, min(blk0 + GRP_BLK, n_blk)):
                        slot = bb - blk0
                        chunks = block_chunks[bb]
                        for ci, (k, t) in enumerate(chunks):
                            st = stp.tile([P, P], F32, name="st")
                            nc.vector.tensor_scalar(
                                out=st[:],
                                in0=iota_s[:],
                                scalar1=lrow_s[:, t:t + 1],
                                scalar2=val_s[:, t:t + 1],
                                op0=mybir.AluOpType.is_equal,
                                op1=mybir.AluOpType.mult,
                            )
                            nc.tensor.matmul(
                                out=psum_g[:, slot, :],
                                lhsT=st[:],
                                rhs=hg_tiles[k][:, t - seg_t0[k], :],
                                start=(ci == 0),
                                stop=False,
                            )
                        nc.tensor.matmul(
                            out=psum_g[:, slot, :],
                            lhsT=rs_s[:, bb * P:(bb + 1) * P],
                            rhs=b_s[:],
                            start=False,
                            stop=True,
                        )
                    # drain the group (only the written slots)
                    n_slot = min(GRP_BLK, n_blk - blk0)
                    ot = otp.tile([P, GRP_BLK, out_dim], F32, name="ot")
                    nc.scalar.activation(
                        ot[:, :n_slot, :], psum_g[:, :n_slot, :],
                        mybir.ActivationFunctionType.Copy)
                    rows_g = min(GRP_BLK * P, shard - blk0 * P)
                    if rows_g == GRP_BLK * P:
                        nc.sync.dma_start(
                            out=out_d[blk0 * P:blk0 * P + rows_g, :].rearrange(
                                "(s p) o -> p s o", p=P),
                            in_=ot[:],
                        )
                    else:
                        for s in range(GRP_BLK):
                            bb = blk0 + s
                            if bb >= n_blk:
                                break
                            rows = min(P, shard - bb * P)
                            nc.sync.dma_start(
                                out=out_d[bb * P:bb * P + rows, :],
                                in_=ot[:rows, s, :],
                            )

    nc.compile()
    return nc


def kernel(X, edge_row, edge_col, edge_val, W, b):
    global LAST_RESULTS
    X = np.asarray(X, dtype=np.float32)
    W = np.asarray(W, dtype=np.float32)
    b = np.asarray(b, dtype=np.float32)

    per_core, layout = _host_prep(edge_row, edge_col, edge_val)

    xt = np.zeros((IN_DIM, NODE_PAD), dtype=ml_dtypes.bfloat16)
    xt[:, :N_NODES] = X.T
    wt = np.ascontiguousarray(W.T).astype(ml_dtypes.bfloat16)
    brow = np.ascontiguousarray(b[None, :])
    iota = np.ascontiguousarray(
        np.tile(np.arange(P, dtype=np.float32), (P, 1)))

    nc = _build_program(layout)

    in_maps = []
    for c in range(N_CORES):
        idx16, lrowf, valf, rs = per_core[c]
        in_maps.append({
            "XT": xt, "WT": wt, "BROW": brow, "IOTA": iota,
            "IDX16": idx16, "LROW": lrowf, "VAL": valf, "RS": rs,
        })

    trace = bool(os.environ.get("GCN_TRACE"))
    kwargs = {}
    if trace:
        _install_ntff_shim()
        tdir = os.environ.get("GCN_TRACE_DIR")
        if tdir:
            os.makedirs(tdir, exist_ok=True)
            kwargs["tmpdir"] = tdir
    LAST_RESULTS = run_bass_kernel_spmd(
        nc, in_maps, core_ids=list(range(N_CORES)), trace=trace, **kwargs,
    )
    out = np.concatenate(
        [LAST_RESULTS.results[c]["OUT"] for c in range(N_CORES)], axis=0)
    return np.ascontiguousarray(out, dtype=np.float32)
